# revision 47
# baseline (speedup 1.0000x reference)
"""CrossAttention kernel for 8x Trainium2 NeuronCores (Bass/Tile).

Reference computation (per batch b):
    q = rope(x @ Wq + bq)  [L, D] -> heads [H, L, HD]
    k = enc @ Wk + bk      [LE, D] -> [H, LE, HD]
    v = enc @ Wv + bv
    out = softmax(q k^T / sqrt(HD)) v  -> concat heads -> @ Wo + bo

Sharding: DP=4 over batch x TP=2 over head-groups. Core c handles batch
(c % 4) and heads [ (c//4)*8 , (c//4)*8+8 ). Each core produces a partial
[L, D] output (row-parallel Wo); host sums the two partials per batch and
adds bo.

Device-side layout choices (all matmuls bf16 inputs, fp32 PSUM accum):
  - host passes x^T and enc^T so the contraction dim is already on
    partitions; no on-device transposes needed anywhere.
  - scores are computed transposed (S^T[m, l]) so that P^T = exp(S^T) is
    directly the moving operand of the ctx^T matmul with V as stationary.
  - softmax skips max-subtraction: scores are ~N(0,1) bounded by ~6 for
    this problem's input distribution, exp is safe in fp32/bf16.
  - 1/sqrt(HD) and the rope pair-sign are baked into host-built cos/sin
    tables; rope pair-swap is a DVE stream_shuffle (mask swaps adjacent
    partitions within each 32-lane quadrant).

Schedule notes (perfetto-profiled):
  - ~3.4us of dummy warm-up matmuls flip the HAM clock gate to 8/8 (2.4
    GHz) during the ~8us engine preamble + first input DMAs, so real
    matmuls run warm from the start.
  - DMA order = PE consumption order; wk/encT/wv are chunked by column
    windows and the projections iterate window-outer (K) / chunk-outer
    (Q) so the PE consumes chunks in arrival order instead of stalling
    on the last chunk of a whole tensor.
  - Wo streams into the SBUF slot of the (dead after Q-proj) Wq tile.
  - exp runs on ACT over [128, 1024] tiles (2 PSUM banks) to amortize the
    ~352-cycle ACT instruction overhead; score matmuls for group g+2 are
    emitted before ctx matmuls of group g so the PE never waits on ACT.
  - the softmax denominator is bf16-folded then partition-reduced AND
    partition-broadcast in one all-ones matmul per 4-chunk group; the
    per-head normalize (reciprocal_approx_fast + scale) is emitted at the
    start of the next head.
  - the second half of the V projection is emitted in quarters between
    attention heads 0-3 so their ACT exp time hides under V matmuls.
"""

import os

import numpy as np
import ml_dtypes

B, L, D = 4, 256, 2048
LE, DE = 2048, 1024
H = 16
HD = D // H  # 128
ROPE_BASE = 10000.0

P = 128
NCORES = 8
HN = H // 2          # heads per core (TP=2)
DC = HN * HD         # 1024 local head dims per core
KCQ = D // P         # 16 k-chunks for Q projection
KCE = DE // P        # 8 k-chunks for K/V projections
MC = LE // P         # 16 key chunks
MW = LE // 512       # 4 key windows for K^T projection
NW = D // 512        # 4 output column windows
LC = L // P          # 2 query-row chunks
NWARM = 36           # PE warm-up matmuls (N=512) at kernel start

BF16 = ml_dtypes.bfloat16

_CACHE = {}
LAST_RESULTS = None  # BassKernelResults of the most recent run (for test.py)


def _build_nc():
    import concourse.bass as bass  # noqa: F401
    import concourse.mybir as mybir
    import concourse.tile as tile
    from concourse import bacc

    f32 = mybir.dt.float32
    bf16 = mybir.dt.bfloat16
    AF = mybir.ActivationFunctionType
    OP = mybir.AluOpType

    nc = bacc.Bacc("TRN2", target_bir_lowering=False, debug=False)

    # xT and encT arrive host-packed in per-partition SBUF order so
    # every load is a contiguous full-rate transfer (their natural
    # layouts have 512B/1KB rows that stream at ~half wire rate)
    xT = nc.dram_tensor("xT", [P, KCQ * L], bf16, kind="ExternalInput").ap()
    encT = nc.dram_tensor("encT", [P, KCE * LE], bf16, kind="ExternalInput").ap()
    wq = nc.dram_tensor("wq", [D, DC], bf16, kind="ExternalInput").ap()
    wk = nc.dram_tensor("wk", [DE, DC], bf16, kind="ExternalInput").ap()
    wv = nc.dram_tensor("wv", [DE, DC], bf16, kind="ExternalInput").ap()
    wo = nc.dram_tensor("wo", [DC, D], bf16, kind="ExternalInput").ap()
    # packed bf16 constants: cos | sin | bvbc  (one DMA)
    CW = L + L + DC
    cstb = nc.dram_tensor("cstb", [P, CW], bf16, kind="ExternalInput").ap()
    # packed f32 constants: bq | bk
    cstf = nc.dram_tensor("cstf", [P, 2 * HN], f32, kind="ExternalInput").ap()
    out = nc.dram_tensor("out", [L, D], bf16, kind="ExternalOutput").ap()

    swap_mask = [i ^ 1 for i in range(32)]

    with tile.TileContext(nc) as tc:
        from contextlib import ExitStack

        with ExitStack() as ctx:
            const = ctx.enter_context(tc.tile_pool(name="const", bufs=1))
            keep = ctx.enter_context(tc.tile_pool(name="keep", bufs=1))
            work = ctx.enter_context(tc.tile_pool(name="work", bufs=2))
            att = ctx.enter_context(tc.tile_pool(name="att", bufs=2))
            ptpool = ctx.enter_context(tc.tile_pool(name="ptp", bufs=3))
            ph1 = ctx.enter_context(tc.tile_pool(name="phase1", bufs=1))
            ps_pp = ctx.enter_context(tc.tile_pool(name="ps_pp", bufs=2, space="PSUM"))
            ps_s = ctx.enter_context(tc.tile_pool(name="ps_s", bufs=2, space="PSUM"))
            ps_c = ctx.enter_context(tc.tile_pool(name="ps_c", bufs=1, space="PSUM"))
            ps_m = ctx.enter_context(tc.tile_pool(name="ps_m", bufs=1, space="PSUM"))

            # --- PE warm-up: ~3.4us of dummy matmuls on a memset tile. The
            # ~8us engine preamble + first DMA latency already delay real
            # matmuls; the warm-up only needs one busy 4096-cycle window to
            # flip the HAM clock gate to 8/8. The all-ones tile doubles as
            # the partition-reduce+broadcast stationary operand for the
            # softmax denominator.
            onesm_sb = const.tile([P, 512], bf16, tag="onesm")
            nc.vector.memset(onesm_sb, 1.0)
            wps = ps_m.tile([P, 512], f32, tag="sbc")
            for i in range(NWARM):
                nc.tensor.matmul(
                    wps, lhsT=onesm_sb[:, 0:P], rhs=onesm_sb, start=True, stop=True
                )

            # --- constants: packed bf16 (cos|sin|bvbc) + tiny f32 (bq|bk)
            cstb_sb = const.tile([P, CW], bf16, tag="cstb")
            cos_sb = cstb_sb[:, 0:L]
            sin_sb = cstb_sb[:, L:2 * L]
            bvbc_sb = cstb_sb[:, 2 * L:]
            cstf_sb = const.tile([P, 2 * HN], f32, tag="cstf")
            bq_sb = cstf_sb[:, 0:HN]
            bk_sb = cstf_sb[:, HN:2 * HN]

            # --- persistent activation tensors ---
            kT_sb = keep.tile([P, HN, LE], bf16, tag="kT")      # K^T per head
            v_sb = keep.tile([P, MC, DC], bf16, tag="v")        # V  [m, d]
            qrot_sb = keep.tile([P, HN, L], bf16, tag="qrot")   # rope(Q)^T
            ctxn_sb = keep.tile([P, HN, L], bf16, tag="ctxn")   # normalized ctx^T

            # --- phase-1 input tiles. DMA order = PE consumption order:
            # cst, xT/wq (progressive row-chunks), then wk/encT/wv chunked
            # by COLUMN windows so K/V-proj tiles unlock as soon as their
            # window lands rather than after the whole tensor.
            # window-major so each window DMA is contiguous on BOTH
            # sides (the dst side otherwise caps the rate at ~196GB/s)
            encT_sb = ph1.tile([P, MW, KCE, 512], bf16, tag="encT")
            wk_sb = ph1.tile([P, KCE, DC], bf16, tag="wk")
            wv_sb = ph1.tile([P, KCE, DC], bf16, tag="wv")
            xT_sb = ph1.tile([P, KCQ, L], bf16, tag="xT")
            wq_sb = ph1.tile([P, KCQ, DC], bf16, tag="wq")

            # DMA order = consumption order. The sync HWDGE ring streams
            # serially at roughly wire rate, so order is priority. Row-chunk
            # loads (contiguous >=2KB rows) run at full rate; the first two
            # encT column windows are strided (1KB rows, ~half rate) but
            # unlock K-proj ~10us earlier than waiting for the full tensor.
            def load_rows(dst, src, k0, n):
                nc.sync.dma_start(
                    dst[:, k0:k0 + n, :],
                    src[k0 * P:(k0 + n) * P, :].rearrange(
                        "(kc p) f -> p kc f", p=P),
                )

            def load_cols(dst, src, c0, c1):
                nc.sync.dma_start(
                    dst[:, :, c0:c1],
                    src[:, c0:c1].rearrange("(kc p) f -> p kc f", p=P),
                )

            nc.sync.dma_start(cstf_sb, cstf)
            # one packed full-rate DMA for xT; wq in four row-chunks —
            # fewer transfers on the serial ring = less fixed overhead,
            # and the warm-up covers until the first chunks land anyway
            nc.sync.dma_start(
                xT_sb, xT.rearrange("p (kc f) -> p kc f", kc=KCQ)
            )
            load_rows(wq_sb, wq, 0, 2)
            load_rows(wq_sb, wq, 2, 2)
            load_rows(wq_sb, wq, 4, 4)
            load_rows(wq_sb, wq, 8, 8)
            def load_encw(w):
                # window w of host-packed encT: contiguous 8KB per partition
                nc.sync.dma_start(
                    encT_sb[:, w, :, :],
                    encT[:, w * KCE * 512:(w + 1) * KCE * 512].rearrange(
                        "p (kc f) -> p kc f", kc=KCE),
                )

            load_rows(wk_sb, wk, 0, KCE)              # full wk, contiguous
            load_encw(0)
            load_encw(1)
            nc.sync.dma_start(cstb_sb, cstb)          # cos/sin: needed at rope

            # --- Q projection, k-chunk-outer so the PE consumes input
            # chunks in DMA arrival order; K-proj window-0 tiles are
            # interleaved into the tail so the PE always has work while Q
            # chunks stream. The 4 concurrent head-pair PSUM accumulators
            # live in the two (idle until attention) ps_s slots: one
            # 512-col bank per head pair, two heads per bank via
            # per-element has_written.
            qpsA = ps_s.tile([P, 1024], f32, tag="sps")
            qpsB = ps_s.tile([P, 1024], f32, tag="sps")
            qps_of = {0: qpsA[:, 0:512], 1: qpsA[:, 512:1024],
                      2: qpsB[:, 0:512], 3: qpsB[:, 512:1024]}

            def q_block(kcs):
                for kc in kcs:
                    for hp in range(HN // 2):
                        for hh in range(2):
                            nc.tensor.matmul(
                                qps_of[hp][:, hh * L:(hh + 1) * L],
                                lhsT=wq_sb[:, kc, (2 * hp + hh) * P:(2 * hp + hh + 1) * P],
                                rhs=xT_sb[:, kc, :],
                                start=(kc == 0 and hh == 0),
                                stop=(kc == KCQ - 1 and hh == 1),
                            )

            def k_tiles(w, hs):
                for h in hs:
                    kps = ps_pp.tile([P, 512], f32, tag="pp")
                    for kc in range(KCE):
                        nc.tensor.matmul(
                            kps,
                            lhsT=wk_sb[:, kc, h * P:(h + 1) * P],
                            rhs=encT_sb[:, w, kc, :],
                            start=(kc == 0),
                            stop=(kc == KCE - 1),
                        )
                    nc.scalar.activation(
                        kT_sb[:, h, w * 512:(w + 1) * 512],
                        kps,
                        AF.Identity,
                        bias=bk_sb[:, h:h + 1],
                    )

            # Q first (its chunks are at the head of the DMA stream and
            # pace the PE from ~9us); K window 0 follows right as encT w0
            # lands.
            q_block(range(0, 16))
            k_tiles(0, range(HN))

            # rope drain of Q
            for hp in range(HN // 2):
                for hh in range(2):
                    h = 2 * hp + hh
                    qf = work.tile([P, L], f32, tag="qf")
                    nc.scalar.activation(
                        qf, qps_of[hp][:, hh * L:(hh + 1) * L], AF.Identity,
                        bias=bq_sb[:, h:h + 1],
                    )
                    qs = work.tile([P, L], f32, tag="qs")
                    nc.vector.stream_shuffle(qs, qf, swap_mask)
                    t1 = work.tile([P, L], f32, tag="t1")
                    nc.vector.tensor_tensor(t1, qf, cos_sb, op=OP.mult)
                    t2 = work.tile([P, L], f32, tag="t2")
                    nc.vector.tensor_tensor(t2, qs, sin_sb, op=OP.mult)
                    nc.vector.tensor_tensor(qrot_sb[:, h, :], t1, t2, op=OP.add)

            # rest of the input stream: upper encT half (contiguous 2KB
            # rows), wv, then Wo into the dead Wq slot (the DMA waits for
            # Q-proj's last read of that slot)
            load_encw(2)
            load_encw(3)
            load_rows(wv_sb, wv, 0, KCE)              # full wv, contiguous
            wo_sb = ph1.tile([P, HN, D], bf16, tag="wq")
            for c in range(4):
                h0 = 2 * c
                nc.sync.dma_start(
                    wo_sb[:, h0:h0 + 2, :],
                    wo[h0 * P:(h0 + 2) * P, :].rearrange(
                        "(h p) n -> p h n", p=P),
                )

            # --- K^T projection windows 1-3 (window 0 interleaved above) ---
            for w in range(1, MW):
                k_tiles(w, range(HN))

            # --- V projection (nh=0 up front; nh=1 in quarters between
            # attention heads 0-3 so their exp time hides under V matmuls)
            def v_proj_chunk(nh, mcs):
                for mc in mcs:
                    vps = ps_pp.tile([P, 512], f32, tag="pp")
                    for kc in range(KCE):
                        nc.tensor.matmul(
                            vps,
                            lhsT=encT_sb[:, mc // 4, kc,
                                         (mc % 4) * P:(mc % 4 + 1) * P],
                            rhs=wv_sb[:, kc, nh * 512:(nh + 1) * 512],
                            start=(kc == 0),
                            stop=(kc == KCE - 1),
                        )
                    nc.vector.tensor_tensor(
                        v_sb[:, mc, nh * 512:(nh + 1) * 512],
                        vps,
                        bvbc_sb[:, nh * 512:(nh + 1) * 512],
                        op=OP.add,
                    )

            v_proj_chunk(0, range(MC))

            # --- attention, software-pipelined across heads ---
            # Per head: score matmuls in 4-chunk groups into 2-bank PSUM
            # tiles, one wide exp per group on ACT, ctx matmuls delayed two
            # groups behind so the PE never waits on ACT. The softmax
            # denominator is bf16-folded (alternating DVE/GPSIMD) to
            # [128, L] per group, then a single all-ones stationary matmul
            # per group both partition-reduces AND broadcasts the running
            # sum to all 128 partitions of a PSUM tile. The normalize of
            # head h (reciprocal + ctx scale, both reading PSUM) is emitted
            # at the start of head h+1 so it frees the ctx/denominator
            # banks well before they are reused.
            NQ = 4                       # key-chunks per exp group
            NG = MC // NQ                # exp groups per head
            state = [dict() for _ in range(HN)]

            def norm(h):
                st = state[h]
                recip = att.tile([P, L], f32, tag="recip")
                nc.vector.reciprocal_approx_fast(recip, st["bigsum"])
                nc.vector.tensor_tensor(
                    ctxn_sb[:, h, :], st["ctxps"], recip, op=OP.mult
                )

            def emit_attention(h, mid_hook=None):
                st = state[h]
                ctxps = ps_c.tile([P, L], f32, tag="ctx")
                bigsum = ps_m.tile([P, L], f32, tag="sbc")
                st.update(ctxps=ctxps, bigsum=bigsum)
                pts = [None] * NG

                def ctx_group(g):
                    pt = pts[g]
                    for q in range(NQ):
                        mc = NQ * g + q
                        nc.tensor.matmul(
                            ctxps,
                            lhsT=v_sb[:, mc, h * P:(h + 1) * P],
                            rhs=pt[:, q * L:(q + 1) * L],
                            start=(mc == 0),
                            stop=(mc == MC - 1),
                        )
                    # denominator: bf16-fold the 4 chunks to [128, L]
                    # (early groups on the slower GPSIMD which has slack;
                    # late groups on DVE so the tail fold is fast), then
                    # one all-ones matmul accumulates the partition-reduced
                    # sum, already broadcast across partitions. The last
                    # group's matmul is deferred into the next head's
                    # stream so the PE never waits on the fold chain.
                    eng = nc.gpsimd if g < 2 else nc.vector
                    tf = att.tile([P, 2 * L], bf16, tag="tf")
                    eng.tensor_tensor(
                        tf, pt[:, 0:2 * L], pt[:, 2 * L:4 * L], op=OP.add
                    )
                    t2 = att.tile([P, L], bf16, tag="t2")
                    eng.tensor_tensor(t2, tf[:, 0:L], tf[:, L:2 * L], op=OP.add)

                    def bs(gg=g, t2=t2):
                        nc.tensor.matmul(
                            bigsum, lhsT=onesm_sb[:, 0:P], rhs=t2,
                            start=(gg == 0), stop=(gg == NG - 1),
                        )
                    if g == NG - 1:
                        st["pending_bigsum"] = bs
                    else:
                        bs()

                def scores_group(g):
                    # 4 key-chunks share one 2-bank PSUM tile; the first
                    # mm per bank sets start=True (clears that bank)
                    sps = ps_s.tile([P, NQ * L], f32, tag="sps")
                    for q in range(NQ):
                        mc = NQ * g + q
                        nc.tensor.matmul(
                            sps[:, q * L:(q + 1) * L],
                            lhsT=kT_sb[:, h, mc * P:(mc + 1) * P],
                            rhs=qrot_sb[:, h, :],
                            start=(q % 2 == 0),
                            stop=(q % 2 == 1),
                        )
                    pt = ptpool.tile([P, NQ * L], bf16, tag="pt")
                    nc.scalar.activation(pt, sps, AF.Exp)
                    pts[g] = pt

                # first two score groups (and their exps) are queued before
                # mid_hook's V-quarter/normalize so ACT cooks exps under
                # independent PE work; ctx stays two groups behind scores
                scores_group(0)
                scores_group(1)
                if mid_hook is not None:
                    mid_hook()
                scores_group(2)
                ctx_group(0)
                scores_group(3)
                ctx_group(1)
                ctx_group(2)
                ctx_group(3)

            # output projection is split into two half-sums over heads:
            # the heads-0..3 half runs interleaved into attention heads
            # 5-7 (their ctxn are final once head 4's mid normalizes head
            # 3), parked as f32 partials in the dead encT slot; the tail
            # then only runs the heads-4..7 half plus one DVE add each.
            osbA = None

            def out_chunk_a(chunks):
                for c in chunks:
                    lc, nw = divmod(c, NW)
                    ops = ps_pp.tile([P, 512], f32, tag="pp")
                    for h in range(HN // 2):
                        nc.tensor.matmul(
                            ops,
                            lhsT=ctxn_sb[:, h, lc * P:(lc + 1) * P],
                            rhs=wo_sb[:, h, nw * 512:(nw + 1) * 512],
                            start=(h == 0),
                            stop=(h == HN // 2 - 1),
                        )
                    nc.vector.tensor_copy(osbA[:, c, :], ops)

            for h in range(HN):
                def mid(hh=h):
                    if hh >= 1:
                        state[hh - 1].pop("pending_bigsum")()
                    if 1 <= hh <= 4:
                        v_proj_chunk(1, range((hh - 1) * 4, hh * 4))
                    if hh >= 1:
                        norm(hh - 1)
                    if hh >= 4:
                        out_chunk_a(range((hh - 4) * 2, (hh - 3) * 2))
                if h == 4:
                    # bf16 partials of the first out-proj half; reuses the
                    # (dead after V-proj) encT SBUF slot
                    osbA = ph1.tile([P, LC * NW, 512], bf16, tag="encT")
                emit_attention(h, mid_hook=mid)
            state[HN - 1].pop("pending_bigsum")()
            norm(HN - 1)

            # --- output projection, heads 4-7 half + combine. Chunks
            # rotate over four PSUM banks (the ctx/denominator banks are
            # free by now) so no chunk waits on a previous chunk's drain.
            def out_psum(c):
                if c % 4 < 2:
                    return ps_pp.tile([P, 512], f32, tag="pp", name="ops")
                if c % 4 == 2:
                    return ps_c.tile([P, 512], f32, tag="ctx", name="ops")
                return ps_m.tile([P, 512], f32, tag="sbc", name="ops")

            for lc in range(LC):
                for nw in range(NW):
                    c = lc * NW + nw
                    ops = out_psum(c)
                    for h in range(HN // 2, HN):
                        nc.tensor.matmul(
                            ops,
                            lhsT=ctxn_sb[:, h, lc * P:(lc + 1) * P],
                            rhs=wo_sb[:, h, nw * 512:(nw + 1) * 512],
                            start=(h == HN // 2),
                            stop=(h == HN - 1),
                        )
                    osb = work.tile([P, 512], bf16, tag="osb", bufs=4)
                    nc.vector.tensor_tensor(osb, ops, osbA[:, c, :], op=OP.add)
                    nc.sync.dma_start(
                        out[lc * P:(lc + 1) * P, nw * 512:(nw + 1) * 512],
                        osb,
                    )

    nc.compile()
    return nc


def _rope_tables():
    half = HD // 2
    inv_freq = 1.0 / (ROPE_BASE ** (np.arange(0, HD, 2, dtype=np.float64) / HD))
    pos = np.arange(L, dtype=np.float64)
    ang = pos[None, :] * inv_freq[:, None]  # [half, L]
    sc = 1.0 / np.sqrt(np.float64(HD))
    cos_t = np.empty((P, L), dtype=np.float32)
    sin_t = np.empty((P, L), dtype=np.float32)
    c = (np.cos(ang) * sc).astype(np.float32)
    s = (np.sin(ang) * sc).astype(np.float32)
    cos_t[0::2, :] = c
    cos_t[1::2, :] = c
    sin_t[0::2, :] = -s
    sin_t[1::2, :] = s
    return cos_t, sin_t


def prepare_in_maps(x, enc, Wq, bq, Wk, bk, Wv, bv, Wo):
    cos_t, sin_t = _rope_tables()

    # per-batch activations and per-head-group weights are shared between
    # cores (DP pairs share weights, TP pairs share activations) — build
    # each distinct array once and alias it into both cores' maps.
    # pack activations into per-partition SBUF order (see _build_nc):
    # xT[p, kc*L+f] = x[b][f, kc*128+p];  encT packed window-major:
    # encT[p, w*KCE*512 + kc*512 + f] = enc[b][w*512+f, kc*128+p]
    xTs = [
        np.ascontiguousarray(
            x[b].reshape(L, KCQ, P).transpose(2, 1, 0).reshape(P, KCQ * L)
        ).astype(BF16)
        for b in range(B)
    ]
    encTs = [
        np.ascontiguousarray(
            enc[b].reshape(MW, 512, KCE, P).transpose(3, 0, 2, 1).reshape(P, KCE * LE)
        ).astype(BF16)
        for b in range(B)
    ]
    grp = []
    for g in range(2):
        sl = slice(g * DC, (g + 1) * DC)
        cstbv = np.concatenate([
            cos_t,
            sin_t,
            np.broadcast_to(bv[sl][None, :], (P, DC)),
        ], axis=1).astype(BF16)
        cstfv = np.concatenate([
            np.ascontiguousarray(bq[sl].reshape(HN, P).T),
            np.ascontiguousarray(bk[sl].reshape(HN, P).T),
        ], axis=1).astype(np.float32)
        grp.append({
            "wq": np.ascontiguousarray(Wq[:, sl]).astype(BF16),
            "wk": np.ascontiguousarray(Wk[:, sl]).astype(BF16),
            "wv": np.ascontiguousarray(Wv[:, sl]).astype(BF16),
            "wo": np.ascontiguousarray(Wo[sl, :]).astype(BF16),
            "cstb": cstbv,
            "cstf": cstfv,
        })

    in_maps = []
    for c in range(NCORES):
        b = c % B
        g = c // B
        in_maps.append({"xT": xTs[b], "encT": encTs[b], **grp[g]})
    return in_maps


def kernel(x, encoder_inputs, Wq, bq, Wk, bk, Wv, bv, Wo, bo):
    global LAST_RESULTS
    from concourse.bass_utils import run_bass_kernel_spmd

    x = np.asarray(x, dtype=np.float32)
    enc = np.asarray(encoder_inputs, dtype=np.float32)
    Wq = np.asarray(Wq, dtype=np.float32)
    Wk = np.asarray(Wk, dtype=np.float32)
    Wv = np.asarray(Wv, dtype=np.float32)
    Wo = np.asarray(Wo, dtype=np.float32)
    bq = np.asarray(bq, dtype=np.float32)
    bk = np.asarray(bk, dtype=np.float32)
    bv = np.asarray(bv, dtype=np.float32)
    bo = np.asarray(bo, dtype=np.float32)

    if "nc" not in _CACHE:
        _CACHE["nc"] = _build_nc()
    nc = _CACHE["nc"]

    in_maps = prepare_in_maps(x, enc, Wq, bq, Wk, bk, Wv, bv, Wo)

    trace = bool(int(os.environ.get("KERNEL_TRACE", "0")))
    try:
        res = run_bass_kernel_spmd(
            nc, in_maps, core_ids=list(range(NCORES)), trace=trace
        )
    except ModuleNotFoundError:
        # NTFF profiling hook unavailable (axon client without antenv hooks)
        res = run_bass_kernel_spmd(
            nc, in_maps, core_ids=list(range(NCORES)), trace=False
        )
    LAST_RESULTS = res

    out = np.empty((B, L, D), dtype=np.float32)
    for b in range(B):
        out[b] = (
            res.results[b]["out"].astype(np.float32)
            + res.results[b + B]["out"].astype(np.float32)
            + bo[None, :]
        )
    return out


# revision 49
# speedup vs baseline: 1.0215x; 1.0215x over previous
"""CrossAttention kernel for 8x Trainium2 NeuronCores (Bass/Tile).

Reference computation (per batch b):
    q = rope(x @ Wq + bq)  [L, D] -> heads [H, L, HD]
    k = enc @ Wk + bk      [LE, D] -> [H, LE, HD]
    v = enc @ Wv + bv
    out = softmax(q k^T / sqrt(HD)) v  -> concat heads -> @ Wo + bo

Sharding: DP=4 over batch x TP=2 over head-groups. Core c handles batch
(c % 4) and heads [ (c//4)*8 , (c//4)*8+8 ). Each core produces a partial
[L, D] output (row-parallel Wo); host sums the two partials per batch and
adds bo.

Device-side layout choices (all matmuls bf16 inputs, fp32 PSUM accum):
  - host passes x^T and enc^T so the contraction dim is already on
    partitions; no on-device transposes needed anywhere.
  - scores are computed transposed (S^T[m, l]) so that P^T = exp(S^T) is
    directly the moving operand of the ctx^T matmul with V as stationary.
  - softmax skips max-subtraction: scores are ~N(0,1) bounded by ~6 for
    this problem's input distribution, exp is safe in fp32/bf16.
  - 1/sqrt(HD) and the rope pair-sign are baked into host-built cos/sin
    tables; rope pair-swap is a DVE stream_shuffle (mask swaps adjacent
    partitions within each 32-lane quadrant).

Schedule notes (perfetto-profiled):
  - ~3.4us of dummy warm-up matmuls flip the HAM clock gate to 8/8 (2.4
    GHz) during the ~8us engine preamble + first input DMAs, so real
    matmuls run warm from the start.
  - DMA order = PE consumption order; wk/encT/wv are chunked by column
    windows and the projections iterate window-outer (K) / chunk-outer
    (Q) so the PE consumes chunks in arrival order instead of stalling
    on the last chunk of a whole tensor.
  - Wo streams into the SBUF slot of the (dead after Q-proj) Wq tile.
  - exp runs on ACT over [128, 1024] tiles (2 PSUM banks) to amortize the
    ~352-cycle ACT instruction overhead; score matmuls for group g+2 are
    emitted before ctx matmuls of group g so the PE never waits on ACT.
  - the softmax denominator is bf16-folded then partition-reduced AND
    partition-broadcast in one all-ones matmul per 4-chunk group; the
    per-head normalize (reciprocal_approx_fast + scale) is emitted at the
    start of the next head.
  - the second half of the V projection is emitted in quarters between
    attention heads 0-3 so their ACT exp time hides under V matmuls.
"""

import os

import numpy as np
import ml_dtypes

B, L, D = 4, 256, 2048
LE, DE = 2048, 1024
H = 16
HD = D // H  # 128
ROPE_BASE = 10000.0

P = 128
NCORES = 8
HN = H // 2          # heads per core (TP=2)
DC = HN * HD         # 1024 local head dims per core
KCQ = D // P         # 16 k-chunks for Q projection
KCE = DE // P        # 8 k-chunks for K/V projections
MC = LE // P         # 16 key chunks
MW = LE // 512       # 4 key windows for K^T projection
NW = D // 512        # 4 output column windows
LC = L // P          # 2 query-row chunks
NWARM = 36           # PE warm-up matmuls (N=512) at kernel start

BF16 = ml_dtypes.bfloat16

_CACHE = {}
LAST_RESULTS = None  # BassKernelResults of the most recent run (for test.py)


def _build_nc():
    import concourse.bass as bass  # noqa: F401
    import concourse.mybir as mybir
    import concourse.tile as tile
    from concourse import bacc

    f32 = mybir.dt.float32
    bf16 = mybir.dt.bfloat16
    AF = mybir.ActivationFunctionType
    OP = mybir.AluOpType

    nc = bacc.Bacc("TRN2", target_bir_lowering=False, debug=False)

    # xT and encT arrive host-packed in per-partition SBUF order so
    # every load is a contiguous full-rate transfer (their natural
    # layouts have 512B/1KB rows that stream at ~half wire rate)
    xT = nc.dram_tensor("xT", [P, KCQ * L], bf16, kind="ExternalInput").ap()
    encT = nc.dram_tensor("encT", [P, KCE * LE], bf16, kind="ExternalInput").ap()
    wq = nc.dram_tensor("wq", [D, DC], bf16, kind="ExternalInput").ap()
    wk = nc.dram_tensor("wk", [DE, DC], bf16, kind="ExternalInput").ap()
    wv = nc.dram_tensor("wv", [DE, DC], bf16, kind="ExternalInput").ap()
    wo = nc.dram_tensor("wo", [DC, D], bf16, kind="ExternalInput").ap()
    # packed bf16 constants: cos | sin | bvbc  (one DMA)
    CW = L + L + DC
    cstb = nc.dram_tensor("cstb", [P, CW], bf16, kind="ExternalInput").ap()
    # packed f32 constants: bq | bk
    cstf = nc.dram_tensor("cstf", [P, 2 * HN], f32, kind="ExternalInput").ap()
    out = nc.dram_tensor("out", [L, D], bf16, kind="ExternalOutput").ap()

    swap_mask = [i ^ 1 for i in range(32)]

    with tile.TileContext(nc) as tc:
        from contextlib import ExitStack

        with ExitStack() as ctx:
            const = ctx.enter_context(tc.tile_pool(name="const", bufs=1))
            keep = ctx.enter_context(tc.tile_pool(name="keep", bufs=1))
            work = ctx.enter_context(tc.tile_pool(name="work", bufs=2))
            att = ctx.enter_context(tc.tile_pool(name="att", bufs=2))
            ptpool = ctx.enter_context(tc.tile_pool(name="ptp", bufs=3))
            ph1 = ctx.enter_context(tc.tile_pool(name="phase1", bufs=1))
            ps_pp = ctx.enter_context(tc.tile_pool(name="ps_pp", bufs=2, space="PSUM"))
            ps_s = ctx.enter_context(tc.tile_pool(name="ps_s", bufs=2, space="PSUM"))
            ps_c = ctx.enter_context(tc.tile_pool(name="ps_c", bufs=1, space="PSUM"))
            ps_m = ctx.enter_context(tc.tile_pool(name="ps_m", bufs=1, space="PSUM"))

            # --- PE warm-up: ~3.4us of dummy matmuls on a memset tile. The
            # ~8us engine preamble + first DMA latency already delay real
            # matmuls; the warm-up only needs one busy 4096-cycle window to
            # flip the HAM clock gate to 8/8. The all-ones tile doubles as
            # the partition-reduce+broadcast stationary operand for the
            # softmax denominator.
            onesm_sb = const.tile([P, 512], bf16, tag="onesm")
            nc.vector.memset(onesm_sb, 1.0)
            wps = ps_m.tile([P, 512], f32, tag="sbc")
            for i in range(NWARM):
                nc.tensor.matmul(
                    wps, lhsT=onesm_sb[:, 0:P], rhs=onesm_sb, start=True, stop=True
                )

            # --- constants: packed bf16 (cos|sin|bvbc) + tiny f32 (bq|bk)
            cstb_sb = const.tile([P, CW], bf16, tag="cstb")
            cos_sb = cstb_sb[:, 0:L]
            sin_sb = cstb_sb[:, L:2 * L]
            bvbc_sb = cstb_sb[:, 2 * L:]
            cstf_sb = const.tile([P, 2 * HN], f32, tag="cstf")
            bq_sb = cstf_sb[:, 0:HN]
            bk_sb = cstf_sb[:, HN:2 * HN]

            # --- persistent activation tensors ---
            kT_sb = keep.tile([P, HN, LE], bf16, tag="kT")      # K^T per head
            v_sb = keep.tile([P, MC, DC], bf16, tag="v")        # V  [m, d]
            qrot_sb = keep.tile([P, HN, L], bf16, tag="qrot")   # rope(Q)^T
            ctxn_sb = keep.tile([P, HN, L], bf16, tag="ctxn")   # normalized ctx^T

            # --- phase-1 input tiles. DMA order = PE consumption order:
            # cst, xT/wq (progressive row-chunks), then wk/encT/wv chunked
            # by COLUMN windows so K/V-proj tiles unlock as soon as their
            # window lands rather than after the whole tensor.
            # window-major so each window DMA is contiguous on BOTH
            # sides (the dst side otherwise caps the rate at ~196GB/s)
            encT_sb = ph1.tile([P, MW, KCE, 512], bf16, tag="encT")
            wk_sb = ph1.tile([P, KCE, DC], bf16, tag="wk")
            wv_sb = ph1.tile([P, KCE, DC], bf16, tag="wv")
            xT_sb = ph1.tile([P, KCQ, L], bf16, tag="xT")
            wq_sb = ph1.tile([P, KCQ, DC], bf16, tag="wq")

            # DMA order = consumption order. The sync HWDGE ring streams
            # serially at roughly wire rate, so order is priority. Row-chunk
            # loads (contiguous >=2KB rows) run at full rate; the first two
            # encT column windows are strided (1KB rows, ~half rate) but
            # unlock K-proj ~10us earlier than waiting for the full tensor.
            def load_rows(dst, src, k0, n):
                nc.sync.dma_start(
                    dst[:, k0:k0 + n, :],
                    src[k0 * P:(k0 + n) * P, :].rearrange(
                        "(kc p) f -> p kc f", p=P),
                )

            def load_cols(dst, src, c0, c1):
                nc.sync.dma_start(
                    dst[:, :, c0:c1],
                    src[:, c0:c1].rearrange("(kc p) f -> p kc f", p=P),
                )

            nc.sync.dma_start(cstf_sb, cstf)
            # one packed full-rate DMA for xT; wq in four row-chunks —
            # fewer transfers on the serial ring = less fixed overhead,
            # and the warm-up covers until the first chunks land anyway
            nc.sync.dma_start(
                xT_sb, xT.rearrange("p (kc f) -> p kc f", kc=KCQ)
            )
            load_rows(wq_sb, wq, 0, 2)
            load_rows(wq_sb, wq, 2, 2)
            load_rows(wq_sb, wq, 4, 4)
            load_rows(wq_sb, wq, 8, 8)
            def load_encw(w):
                # window w of host-packed encT: contiguous 8KB per partition
                nc.sync.dma_start(
                    encT_sb[:, w, :, :],
                    encT[:, w * KCE * 512:(w + 1) * KCE * 512].rearrange(
                        "p (kc f) -> p kc f", kc=KCE),
                )

            load_rows(wk_sb, wk, 0, KCE)              # full wk, contiguous
            load_encw(0)
            load_encw(1)
            nc.sync.dma_start(cstb_sb, cstb)          # cos/sin: needed at rope

            # --- Q projection, k-chunk-outer so the PE consumes input
            # chunks in DMA arrival order; K-proj window-0 tiles are
            # interleaved into the tail so the PE always has work while Q
            # chunks stream. The 4 concurrent head-pair PSUM accumulators
            # live in the two (idle until attention) ps_s slots: one
            # 512-col bank per head pair, two heads per bank via
            # per-element has_written.
            qpsA = ps_s.tile([P, 1024], f32, tag="sps")
            qpsB = ps_s.tile([P, 1024], f32, tag="sps")
            qps_of = {0: qpsA[:, 0:512], 1: qpsA[:, 512:1024],
                      2: qpsB[:, 0:512], 3: qpsB[:, 512:1024]}

            def q_block(kcs):
                for kc in kcs:
                    for hp in range(HN // 2):
                        for hh in range(2):
                            nc.tensor.matmul(
                                qps_of[hp][:, hh * L:(hh + 1) * L],
                                lhsT=wq_sb[:, kc, (2 * hp + hh) * P:(2 * hp + hh + 1) * P],
                                rhs=xT_sb[:, kc, :],
                                start=(kc == 0 and hh == 0),
                                stop=(kc == KCQ - 1 and hh == 1),
                            )

            def k_tiles(w, hs):
                for h in hs:
                    kps = ps_pp.tile([P, 512], f32, tag="pp")
                    for kc in range(KCE):
                        nc.tensor.matmul(
                            kps,
                            lhsT=wk_sb[:, kc, h * P:(h + 1) * P],
                            rhs=encT_sb[:, w, kc, :],
                            start=(kc == 0),
                            stop=(kc == KCE - 1),
                        )
                    nc.scalar.activation(
                        kT_sb[:, h, w * 512:(w + 1) * 512],
                        kps,
                        AF.Identity,
                        bias=bk_sb[:, h:h + 1],
                    )

            # Q first (its chunks are at the head of the DMA stream and
            # pace the PE from ~9us); K window 0 follows right as encT w0
            # lands.
            q_block(range(0, 16))
            # second warm-up burst: lands exactly in the PE-queue position
            # where Q-proj is exhausted but encT window 0 hasn't arrived,
            # keeping the HAM clock warm into K-proj
            for i in range(12):
                nc.tensor.matmul(
                    wps, lhsT=onesm_sb[:, 0:P], rhs=onesm_sb, start=True, stop=True
                )
            k_tiles(0, range(HN))

            # rope drain of Q
            for hp in range(HN // 2):
                for hh in range(2):
                    h = 2 * hp + hh
                    qf = work.tile([P, L], f32, tag="qf")
                    nc.scalar.activation(
                        qf, qps_of[hp][:, hh * L:(hh + 1) * L], AF.Identity,
                        bias=bq_sb[:, h:h + 1],
                    )
                    qs = work.tile([P, L], f32, tag="qs")
                    nc.vector.stream_shuffle(qs, qf, swap_mask)
                    t1 = work.tile([P, L], f32, tag="t1")
                    nc.vector.tensor_tensor(t1, qf, cos_sb, op=OP.mult)
                    t2 = work.tile([P, L], f32, tag="t2")
                    nc.vector.tensor_tensor(t2, qs, sin_sb, op=OP.mult)
                    nc.vector.tensor_tensor(qrot_sb[:, h, :], t1, t2, op=OP.add)

            # rest of the input stream: upper encT half (contiguous 2KB
            # rows), wv, then Wo into the dead Wq slot (the DMA waits for
            # Q-proj's last read of that slot)
            load_encw(2)
            load_encw(3)
            load_rows(wv_sb, wv, 0, KCE)              # full wv, contiguous
            wo_sb = ph1.tile([P, HN, D], bf16, tag="wq")
            for c in range(4):
                h0 = 2 * c
                nc.sync.dma_start(
                    wo_sb[:, h0:h0 + 2, :],
                    wo[h0 * P:(h0 + 2) * P, :].rearrange(
                        "(h p) n -> p h n", p=P),
                )

            # --- K^T projection windows 1-3 (window 0 interleaved above) ---
            for w in range(1, MW):
                k_tiles(w, range(HN))

            # --- V projection (nh=0 up front; nh=1 in quarters between
            # attention heads 0-3 so their exp time hides under V matmuls)
            def v_proj_chunk(nh, mcs):
                for mc in mcs:
                    vps = ps_pp.tile([P, 512], f32, tag="pp")
                    for kc in range(KCE):
                        nc.tensor.matmul(
                            vps,
                            lhsT=encT_sb[:, mc // 4, kc,
                                         (mc % 4) * P:(mc % 4 + 1) * P],
                            rhs=wv_sb[:, kc, nh * 512:(nh + 1) * 512],
                            start=(kc == 0),
                            stop=(kc == KCE - 1),
                        )
                    nc.vector.tensor_tensor(
                        v_sb[:, mc, nh * 512:(nh + 1) * 512],
                        vps,
                        bvbc_sb[:, nh * 512:(nh + 1) * 512],
                        op=OP.add,
                    )

            v_proj_chunk(0, range(MC))

            # --- attention, software-pipelined across heads ---
            # Per head: score matmuls in 4-chunk groups into 2-bank PSUM
            # tiles, one wide exp per group on ACT, ctx matmuls delayed two
            # groups behind so the PE never waits on ACT. The softmax
            # denominator is bf16-folded (alternating DVE/GPSIMD) to
            # [128, L] per group, then a single all-ones stationary matmul
            # per group both partition-reduces AND broadcasts the running
            # sum to all 128 partitions of a PSUM tile. The normalize of
            # head h (reciprocal + ctx scale, both reading PSUM) is emitted
            # at the start of head h+1 so it frees the ctx/denominator
            # banks well before they are reused.
            NQ = 4                       # key-chunks per exp group
            NG = MC // NQ                # exp groups per head
            state = [dict() for _ in range(HN)]

            def norm(h):
                st = state[h]
                recip = att.tile([P, L], f32, tag="recip")
                nc.vector.reciprocal_approx_fast(recip, st["bigsum"])
                nc.vector.tensor_tensor(
                    ctxn_sb[:, h, :], st["ctxps"], recip, op=OP.mult
                )

            def emit_attention(h, mid_hook=None):
                st = state[h]
                ctxps = ps_c.tile([P, L], f32, tag="ctx")
                bigsum = ps_m.tile([P, L], f32, tag="sbc")
                st.update(ctxps=ctxps, bigsum=bigsum)
                pts = [None] * NG

                def ctx_group(g):
                    pt = pts[g]
                    for q in range(NQ):
                        mc = NQ * g + q
                        nc.tensor.matmul(
                            ctxps,
                            lhsT=v_sb[:, mc, h * P:(h + 1) * P],
                            rhs=pt[:, q * L:(q + 1) * L],
                            start=(mc == 0),
                            stop=(mc == MC - 1),
                        )
                    # denominator: bf16-fold the 4 chunks to [128, L]
                    # (early groups on the slower GPSIMD which has slack;
                    # late groups on DVE so the tail fold is fast), then
                    # one all-ones matmul accumulates the partition-reduced
                    # sum, already broadcast across partitions. The last
                    # group's matmul is deferred into the next head's
                    # stream so the PE never waits on the fold chain.
                    eng = nc.gpsimd if g < 2 else nc.vector
                    tf = att.tile([P, 2 * L], bf16, tag="tf")
                    eng.tensor_tensor(
                        tf, pt[:, 0:2 * L], pt[:, 2 * L:4 * L], op=OP.add
                    )
                    t2 = att.tile([P, L], bf16, tag="t2")
                    eng.tensor_tensor(t2, tf[:, 0:L], tf[:, L:2 * L], op=OP.add)

                    def bs(gg=g, t2=t2):
                        nc.tensor.matmul(
                            bigsum, lhsT=onesm_sb[:, 0:P], rhs=t2,
                            start=(gg == 0), stop=(gg == NG - 1),
                        )
                    if g == NG - 1:
                        st["pending_bigsum"] = bs
                    else:
                        bs()

                def scores_group(g):
                    # 4 key-chunks share one 2-bank PSUM tile; the first
                    # mm per bank sets start=True (clears that bank)
                    sps = ps_s.tile([P, NQ * L], f32, tag="sps")
                    for q in range(NQ):
                        mc = NQ * g + q
                        nc.tensor.matmul(
                            sps[:, q * L:(q + 1) * L],
                            lhsT=kT_sb[:, h, mc * P:(mc + 1) * P],
                            rhs=qrot_sb[:, h, :],
                            start=(q % 2 == 0),
                            stop=(q % 2 == 1),
                        )
                    pt = ptpool.tile([P, NQ * L], bf16, tag="pt")
                    nc.scalar.activation(pt, sps, AF.Exp)
                    pts[g] = pt

                # first two score groups (and their exps) are queued before
                # mid_hook's V-quarter/normalize so ACT cooks exps under
                # independent PE work; ctx stays two groups behind scores
                scores_group(0)
                scores_group(1)
                if mid_hook is not None:
                    mid_hook()
                scores_group(2)
                ctx_group(0)
                scores_group(3)
                ctx_group(1)
                ctx_group(2)
                ctx_group(3)

            # output projection is split into two half-sums over heads:
            # the heads-0..3 half runs interleaved into attention heads
            # 5-7 (their ctxn are final once head 4's mid normalizes head
            # 3), parked as f32 partials in the dead encT slot; the tail
            # then only runs the heads-4..7 half plus one DVE add each.
            osbA = None

            def out_chunk_a(chunks):
                for c in chunks:
                    lc, nw = divmod(c, NW)
                    ops = ps_pp.tile([P, 512], f32, tag="pp")
                    for h in range(HN // 2):
                        nc.tensor.matmul(
                            ops,
                            lhsT=ctxn_sb[:, h, lc * P:(lc + 1) * P],
                            rhs=wo_sb[:, h, nw * 512:(nw + 1) * 512],
                            start=(h == 0),
                            stop=(h == HN // 2 - 1),
                        )
                    nc.vector.tensor_copy(osbA[:, c, :], ops)

            for h in range(HN):
                def mid(hh=h):
                    if hh >= 1:
                        state[hh - 1].pop("pending_bigsum")()
                    if 1 <= hh <= 4:
                        v_proj_chunk(1, range((hh - 1) * 4, hh * 4))
                    if hh >= 1:
                        norm(hh - 1)
                    if hh == 5:
                        out_chunk_a(range(0, 3))
                    elif hh == 6:
                        out_chunk_a(range(3, 6))
                    elif hh == 7:
                        out_chunk_a(range(6, 8))
                if h == 5:
                    # bf16 partials of the first out-proj half; reuses the
                    # (dead after V-proj) encT SBUF slot
                    osbA = ph1.tile([P, LC * NW, 512], bf16, tag="encT")
                emit_attention(h, mid_hook=mid)
            state[HN - 1].pop("pending_bigsum")()
            norm(HN - 1)

            # --- output projection, heads 4-7 half + combine. Chunks
            # rotate over four PSUM banks (the ctx/denominator banks are
            # free by now) so no chunk waits on a previous chunk's drain.
            def out_psum(c):
                if c % 4 < 2:
                    return ps_pp.tile([P, 512], f32, tag="pp", name="ops")
                if c % 4 == 2:
                    return ps_c.tile([P, 512], f32, tag="ctx", name="ops")
                return ps_m.tile([P, 512], f32, tag="sbc", name="ops")

            for lc in range(LC):
                for nw in range(NW):
                    c = lc * NW + nw
                    ops = out_psum(c)
                    for h in range(HN // 2, HN):
                        nc.tensor.matmul(
                            ops,
                            lhsT=ctxn_sb[:, h, lc * P:(lc + 1) * P],
                            rhs=wo_sb[:, h, nw * 512:(nw + 1) * 512],
                            start=(h == HN // 2),
                            stop=(h == HN - 1),
                        )
                    osb = work.tile([P, 512], bf16, tag="osb", bufs=4)
                    nc.vector.tensor_tensor(osb, ops, osbA[:, c, :], op=OP.add)
                    nc.sync.dma_start(
                        out[lc * P:(lc + 1) * P, nw * 512:(nw + 1) * 512],
                        osb,
                    )

    nc.compile()
    return nc


def _rope_tables():
    half = HD // 2
    inv_freq = 1.0 / (ROPE_BASE ** (np.arange(0, HD, 2, dtype=np.float64) / HD))
    pos = np.arange(L, dtype=np.float64)
    ang = pos[None, :] * inv_freq[:, None]  # [half, L]
    sc = 1.0 / np.sqrt(np.float64(HD))
    cos_t = np.empty((P, L), dtype=np.float32)
    sin_t = np.empty((P, L), dtype=np.float32)
    c = (np.cos(ang) * sc).astype(np.float32)
    s = (np.sin(ang) * sc).astype(np.float32)
    cos_t[0::2, :] = c
    cos_t[1::2, :] = c
    sin_t[0::2, :] = -s
    sin_t[1::2, :] = s
    return cos_t, sin_t


def prepare_in_maps(x, enc, Wq, bq, Wk, bk, Wv, bv, Wo):
    cos_t, sin_t = _rope_tables()

    # per-batch activations and per-head-group weights are shared between
    # cores (DP pairs share weights, TP pairs share activations) — build
    # each distinct array once and alias it into both cores' maps.
    # pack activations into per-partition SBUF order (see _build_nc):
    # xT[p, kc*L+f] = x[b][f, kc*128+p];  encT packed window-major:
    # encT[p, w*KCE*512 + kc*512 + f] = enc[b][w*512+f, kc*128+p]
    xTs = [
        np.ascontiguousarray(
            x[b].reshape(L, KCQ, P).transpose(2, 1, 0).reshape(P, KCQ * L)
        ).astype(BF16)
        for b in range(B)
    ]
    encTs = [
        np.ascontiguousarray(
            enc[b].reshape(MW, 512, KCE, P).transpose(3, 0, 2, 1).reshape(P, KCE * LE)
        ).astype(BF16)
        for b in range(B)
    ]
    grp = []
    for g in range(2):
        sl = slice(g * DC, (g + 1) * DC)
        cstbv = np.concatenate([
            cos_t,
            sin_t,
            np.broadcast_to(bv[sl][None, :], (P, DC)),
        ], axis=1).astype(BF16)
        cstfv = np.concatenate([
            np.ascontiguousarray(bq[sl].reshape(HN, P).T),
            np.ascontiguousarray(bk[sl].reshape(HN, P).T),
        ], axis=1).astype(np.float32)
        grp.append({
            "wq": np.ascontiguousarray(Wq[:, sl]).astype(BF16),
            "wk": np.ascontiguousarray(Wk[:, sl]).astype(BF16),
            "wv": np.ascontiguousarray(Wv[:, sl]).astype(BF16),
            "wo": np.ascontiguousarray(Wo[sl, :]).astype(BF16),
            "cstb": cstbv,
            "cstf": cstfv,
        })

    in_maps = []
    for c in range(NCORES):
        b = c % B
        g = c // B
        in_maps.append({"xT": xTs[b], "encT": encTs[b], **grp[g]})
    return in_maps


def kernel(x, encoder_inputs, Wq, bq, Wk, bk, Wv, bv, Wo, bo):
    global LAST_RESULTS
    from concourse.bass_utils import run_bass_kernel_spmd

    x = np.asarray(x, dtype=np.float32)
    enc = np.asarray(encoder_inputs, dtype=np.float32)
    Wq = np.asarray(Wq, dtype=np.float32)
    Wk = np.asarray(Wk, dtype=np.float32)
    Wv = np.asarray(Wv, dtype=np.float32)
    Wo = np.asarray(Wo, dtype=np.float32)
    bq = np.asarray(bq, dtype=np.float32)
    bk = np.asarray(bk, dtype=np.float32)
    bv = np.asarray(bv, dtype=np.float32)
    bo = np.asarray(bo, dtype=np.float32)

    if "nc" not in _CACHE:
        _CACHE["nc"] = _build_nc()
    nc = _CACHE["nc"]

    in_maps = prepare_in_maps(x, enc, Wq, bq, Wk, bk, Wv, bv, Wo)

    trace = bool(int(os.environ.get("KERNEL_TRACE", "0")))
    try:
        res = run_bass_kernel_spmd(
            nc, in_maps, core_ids=list(range(NCORES)), trace=trace
        )
    except ModuleNotFoundError:
        # NTFF profiling hook unavailable (axon client without antenv hooks)
        res = run_bass_kernel_spmd(
            nc, in_maps, core_ids=list(range(NCORES)), trace=False
        )
    LAST_RESULTS = res

    out = np.empty((B, L, D), dtype=np.float32)
    for b in range(B):
        out[b] = (
            res.results[b]["out"].astype(np.float32)
            + res.results[b + B]["out"].astype(np.float32)
            + bo[None, :]
        )
    return out
